# revision 12
# baseline (speedup 1.0000x reference)
"""Exaone4 attention kernel for 8 Trainium2 NeuronCores.

Sharding: tensor-parallel over heads (TP=8). Core i owns query heads
4i..4i+3 and kv head i (one GQA group), processes both batch elements,
and computes a row-parallel partial of the output projection; the host
sums the 8 partials.

Shapes (hardcoded): B=2, S=2048, H=4096, NH=32, NKV=8, D=128,
WINDOW=1024, eps=1e-5, theta=10000.
"""

import os
import sys

for _p in ("/opt/trn_rl_repo",):
    if _p not in sys.path and os.path.isdir(_p):
        sys.path.insert(0, _p)

import numpy as np

B, S, H = 2, 2048, 4096
NH, NKV, D = 32, 8, 128
WINDOW = 1024
EPS = 1e-5
THETA = 10000.0

NCORES = 8
HPC = NH // NCORES          # query heads per core = 4
QW = HPC * D                # q-proj cols per core = 512
CH = 256                    # QKV sequence chunk (PSUM-bank friendly)
NCH = S // CH               # 16 chunks
HC = H // 128               # 32 contraction chunks
NEG = -1.0e30

_CACHE = {}


def _build():
    import concourse.bass as bass
    import concourse.tile as tile
    from concourse import mybir, bacc
    from concourse.tile import add_dep_helper

    F32 = mybir.dt.float32
    F32R = mybir.dt.float32r
    EXP = mybir.ActivationFunctionType.Exp
    SQRT = mybir.ActivationFunctionType.Sqrt

    nc = bacc.Bacc("TRN2", target_bir_lowering=False, debug=False)

    hsT = nc.dram_tensor("hsT", [B, H, S], F32R, kind="ExternalInput")
    wq_s = nc.dram_tensor("wq_s", [H, QW], F32R, kind="ExternalInput")
    wk_s = nc.dram_tensor("wk_s", [H, D], F32R, kind="ExternalInput")
    wv_s = nc.dram_tensor("wv_s", [H, D], F32R, kind="ExternalInput")
    wo_s = nc.dram_tensor("wo_s", [QW, H], F32R, kind="ExternalInput")
    cosT = nc.dram_tensor("cosT", [D, S], F32, kind="ExternalInput")
    sinT = nc.dram_tensor("sinT", [D, S], F32, kind="ExternalInput")
    wrow_q = nc.dram_tensor("wrow_q", [1, D], F32R, kind="ExternalInput")
    wrow_k = nc.dram_tensor("wrow_k", [1, D], F32R, kind="ExternalInput")
    ones128 = nc.dram_tensor("ones128", [128, 1], F32R, kind="ExternalInput")
    ones_col = nc.dram_tensor("ones_col", [1, 128], F32R, kind="ExternalInput")
    protT = nc.dram_tensor("protT", [128, 128], F32R, kind="ExternalInput")
    idn = nc.dram_tensor("idn", [128, 128], F32, kind="ExternalInput")
    mask_c = nc.dram_tensor("mask_c", [128, 128], F32, kind="ExternalInput")
    mask_w = nc.dram_tensor("mask_w", [128, 128], F32, kind="ExternalInput")
    out_part = nc.dram_tensor("out_part", [B, S, H], F32, kind="ExternalOutput")

    DEBUG = bool(os.environ.get("BASS_KERNEL_DEBUG"))
    if DEBUG:
        dbg_k = nc.dram_tensor("dbg_k", [128, S], F32, kind="ExternalOutput")
        dbg_v = nc.dram_tensor("dbg_v", [128, S // 128, 128], F32,
                               kind="ExternalOutput")
        dbg_q = nc.dram_tensor("dbg_q", [HPC, 128, S], F32,
                               kind="ExternalOutput")
        dbg_a = nc.dram_tensor("dbg_a", [HPC, 128, S], F32,
                               kind="ExternalOutput")
        dbg_raw = nc.dram_tensor("dbg_raw", [6, 128, CH], F32,
                                 kind="ExternalOutput")
        dbg_pr = nc.dram_tensor("dbg_pr", [12, 128, 512], F32,
                                kind="ExternalOutput")

    with tile.TileContext(nc) as tc, \
         nc.allow_low_precision(reason="deliberate fp32r matmul pipeline"):
        with tc.tile_pool(name="consts", bufs=1) as consts, \
             tc.tile_pool(name="dram", bufs=2, space="DRAM") as dram:
            cos_sb = consts.tile([D, S], F32)
            nc.sync.dma_start(cos_sb, cosT.ap())
            sin_sb = consts.tile([D, S], F32)
            nc.sync.dma_start(sin_sb, sinT.ap())
            wq_row = consts.tile([1, D], F32R)
            nc.sync.dma_start(wq_row, wrow_q.ap())
            wk_row = consts.tile([1, D], F32R)
            nc.sync.dma_start(wk_row, wrow_k.ap())
            on128 = consts.tile([128, 1], F32R)
            nc.sync.dma_start(on128, ones128.ap())
            oncol = consts.tile([1, 128], F32R)
            nc.sync.dma_start(oncol, ones_col.ap())
            prot = consts.tile([128, 128], F32R)
            nc.sync.dma_start(prot, protT.ap())
            iden = consts.tile([128, 128], F32)
            nc.sync.dma_start(iden, idn.ap())
            mc = consts.tile([128, 128], F32)
            nc.sync.dma_start(mc, mask_c.ap())
            mw = consts.tile([128, 128], F32)
            nc.sync.dma_start(mw, mask_w.ap())
            eps_t = consts.tile([1, 1], F32)
            nc.vector.memset(eps_t, EPS)

            scr = [dram.tile([HPC, 128, S], F32R, tag="attn_scr",
                             name=f"attn_scr{b}") for b in range(B)]

            # ---------------- phases A+B: QKV + norm/rope + attention ----
            with tc.tile_pool(name="wqkv", bufs=1) as wp, \
                 tc.tile_pool(name="kv", bufs=1) as kvp, \
                 tc.tile_pool(name="work", bufs=2) as wrk, \
                 tc.tile_pool(name="hs", bufs=3) as hsp, \
                 tc.tile_pool(name="probs", bufs=3) as prp, \
                 tc.tile_pool(name="qf", bufs=2) as qfp, \
                 tc.tile_pool(name="ps_qkv", bufs=1, space="PSUM") as pq, \
                 tc.tile_pool(name="ps_s", bufs=2, space="PSUM") as pss, \
                 tc.tile_pool(name="ps_o", bufs=1, space="PSUM") as pso, \
                 tc.tile_pool(name="ps_sum", bufs=1, space="PSUM") as psum_p, \
                 tc.tile_pool(name="ps_aux", bufs=1, space="PSUM") as paux:

                wq_sb = wp.tile([128, HC, QW], F32R)
                nc.sync.dma_start(
                    wq_sb, wq_s.ap().rearrange("(o p) c -> p o c", p=128))
                wk_sb = wp.tile([128, HC, D], F32R)
                nc.sync.dma_start(
                    wk_sb, wk_s.ap().rearrange("(o p) c -> p o c", p=128))
                wv_sb = wp.tile([128, HC, D], F32R)
                nc.sync.dma_start(
                    wv_sb, wv_s.ap().rearrange("(o p) c -> p o c", p=128))

                for b in range(B):
                    # K in [D, S]; V in [S, D] (by 128-row tiles)
                    k_full = kvp.tile([128, S], F32R, tag="k_full")
                    v_full = kvp.tile([128, S // 128, 128], F32R, tag="v_full")

                    for qc in range(S // 512):
                        qfin = []
                        for half in range(2):
                            sc2 = 2 * qc + half
                            s0 = CH * sc2
                            # ---- QKV projection for this 256-chunk
                            # NOTE: a matmul with start=True clears the
                            # has_written bits of the WHOLE psum bank, so for
                            # the two 256-wide blocks sharing a bank only the
                            # even block seeds (start=True) and the odd block
                            # must issue after it at hc==0 (dep edge below).
                            qkv_ps = pq.tile([128, 6, CH], F32, tag="qkv")
                            seed_mm = {}
                            for hc in range(HC):
                                ht = hsp.tile([128, CH], F32R, tag="hst")
                                nc.sync.dma_start(
                                    ht, hsT.ap()[b, 128 * hc:128 * (hc + 1),
                                                 s0:s0 + CH])
                                for blk in range(6):
                                    if blk < HPC:
                                        lhs = wq_sb[:, hc,
                                                    128 * blk:128 * (blk + 1)]
                                    elif blk == HPC:
                                        lhs = wk_sb[:, hc, :]
                                    else:
                                        lhs = wv_sb[:, hc, :]
                                    mm = nc.tensor.matmul(
                                        qkv_ps[:, blk, :], lhs, ht,
                                        start=(hc == 0 and blk % 2 == 0),
                                        stop=(hc == HC - 1),
                                        skip_group_check=True)
                                    if hc == 0:
                                        if blk % 2 == 0:
                                            seed_mm[blk // 2] = mm.ins
                                        else:
                                            add_dep_helper(
                                                mm.ins, seed_mm[blk // 2],
                                                sync=False,
                                                reason="psum bank seed order")

                            # ---- norm + rope for the 4 q blocks and k
                            for blk in range(HPC + 1):
                                is_q = blk < HPC
                                raw = wrk.tile([128, CH], F32, tag="raw")
                                nc.scalar.copy(raw, qkv_ps[:, blk, :])
                                sq = wrk.tile([128, CH], F32R, tag="sq")
                                nc.vector.tensor_mul(sq, raw, raw)
                                ssq = paux.tile([1, CH], F32, tag="aux")
                                nc.tensor.matmul(ssq, on128, sq,
                                                 start=True, stop=True)
                                rms = wrk.tile([1, CH], F32, tag="rms")
                                nc.scalar.activation(rms, ssq, SQRT,
                                                     bias=eps_t, scale=1.0 / D)
                                rinv = wrk.tile([1, CH], F32R, tag="rinv")
                                nc.vector.reciprocal(rinv, rms)
                                wrow = wq_row if is_q else wk_row
                                wtil = paux.tile([128, CH], F32, tag="aux")
                                nc.tensor.matmul(wtil, wrow, rinv,
                                                 start=True, stop=True)
                                qhat = wrk.tile([128, CH], F32R, tag="qhat")
                                nc.vector.tensor_mul(qhat, wtil, raw)
                                rot = paux.tile([128, CH], F32, tag="aux")
                                nc.tensor.matmul(rot, prot, qhat,
                                                 start=True, stop=True)
                                t1 = wrk.tile([128, CH], F32, tag="t1")
                                nc.vector.tensor_mul(t1, qhat,
                                                     cos_sb[:, s0:s0 + CH])
                                t2 = wrk.tile([128, CH], F32, tag="t2")
                                nc.vector.tensor_mul(t2, rot,
                                                     sin_sb[:, s0:s0 + CH])
                                if is_q:
                                    if half == 0:
                                        qf = qfp.tile([128, 512], F32R,
                                                      tag=f"qfin{blk}")
                                        qfin.append(qf)
                                    nc.vector.tensor_add(
                                        qfin[blk][:, CH * half:CH * (half + 1)],
                                        t1, t2)
                                else:
                                    nc.vector.tensor_add(
                                        k_full[:, s0:s0 + CH], t1, t2)
                                if DEBUG and b == 0 and sc2 == 4:
                                    nc.sync.dma_start(
                                        dbg_raw.ap()[blk], raw)

                            # ---- V: copy out and transpose to [S, D]
                            vt_sb = wrk.tile([128, CH], F32, tag="vt")
                            nc.scalar.copy(vt_sb, qkv_ps[:, HPC + 1, :])
                            for j in range(CH // 128):
                                tp = paux.tile([128, 128], F32, tag="aux")
                                nc.tensor.transpose(
                                    tp, vt_sb[:, 128 * j:128 * (j + 1)], iden)
                                nc.scalar.copy(
                                    v_full[:, 2 * sc2 + j, :], tp)

                        # ---- attention for query chunk qc (512 queries)
                        s0q = 512 * qc
                        kis = list(range(max(0, 4 * qc - 8), 4 * qc + 4))
                        for h in range(HPC):
                            o_ps = pso.tile([128, 512], F32, tag="o")
                            sum_ps = psum_p.tile([1, 512], F32, tag="sum")
                            for i, ki in enumerate(kis):
                                first, last = (i == 0), (i == len(kis) - 1)
                                s_ps = pss.tile([128, 512], F32, tag="s")
                                nc.tensor.matmul(
                                    s_ps,
                                    k_full[:, 128 * ki:128 * (ki + 1)],
                                    qfin[h], start=True, stop=True)
                                delta = 512 * qc - 128 * ki
                                j = None
                                if delta <= 0:
                                    j = -delta // 128
                                    nc.vector.tensor_add(
                                        s_ps[:, 128 * j:128 * (j + 1)],
                                        s_ps[:, 128 * j:128 * (j + 1)], mc)
                                elif delta >= 640:
                                    j = (1024 - delta) // 128
                                    nc.vector.tensor_add(
                                        s_ps[:, 128 * j:128 * (j + 1)],
                                        s_ps[:, 128 * j:128 * (j + 1)], mw)
                                pr = prp.tile([128, 512], F32R, tag="pr")
                                nc.scalar.activation(pr, s_ps, EXP)
                                if delta <= 0 and j is not None and j > 0:
                                    nc.gpsimd.memset(
                                        pr[:, :128 * j].bitcast(F32), 0.0)
                                if delta >= 640 and j is not None and j < 3:
                                    nc.gpsimd.memset(
                                        pr[:, 128 * (j + 1):].bitcast(F32),
                                        0.0)
                                if DEBUG and b == 0 and h == 0 and qc == 2:
                                    nc.sync.dma_start(
                                        dbg_pr.ap()[i], pr.bitcast(F32))
                                nc.tensor.matmul(sum_ps, on128, pr,
                                                 start=first, stop=last)
                                nc.tensor.matmul(o_ps, v_full[:, ki, :], pr,
                                                 start=first, stop=last)
                            rsum = wrk.tile([1, 512], F32R, tag="rsum")
                            nc.vector.reciprocal(rsum, sum_ps)
                            bc = paux.tile([128, 512], F32, tag="aux")
                            nc.tensor.matmul(bc, oncol, rsum,
                                             start=True, stop=True)
                            a_un = wrk.tile([128, 512], F32, tag="a_un")
                            nc.scalar.copy(a_un, o_ps)
                            a_fin = wrk.tile([128, 512], F32R, tag="a_fin")
                            nc.vector.tensor_mul(a_fin, bc, a_un)
                            nc.sync.dma_start(
                                scr[b][h, :, s0q:s0q + 512], a_fin)
                            if DEBUG and b == 0:
                                nc.sync.dma_start(
                                    dbg_a.ap()[h, :, s0q:s0q + 512],
                                    a_fin.bitcast(F32))
                                nc.sync.dma_start(
                                    dbg_q.ap()[h, :, s0q:s0q + 512],
                                    qfin[h].bitcast(F32))

                    if DEBUG and b == 0:
                        nc.sync.dma_start(dbg_k.ap(), k_full.bitcast(F32))
                        nc.sync.dma_start(dbg_v.ap(), v_full.bitcast(F32))

            # ---------------- phase C: output projection -----------------
            with tc.tile_pool(name="wo", bufs=1) as wop, \
                 tc.tile_pool(name="at", bufs=4) as atp, \
                 tc.tile_pool(name="ostg", bufs=4) as ost, \
                 tc.tile_pool(name="ps_c", bufs=4, space="PSUM") as pc:
                wo_sb = wop.tile([128, QW // 128, H], F32R)
                nc.sync.dma_start(
                    wo_sb, wo_s.ap().rearrange("(o p) c -> p o c", p=128))
                for b in range(B):
                    for st in range(S // 128):
                        a_t = []
                        for r in range(QW // 128):
                            at = atp.tile([128, 128], F32R, tag=f"at{r}")
                            nc.sync.dma_start(
                                at, scr[b][r, :, 128 * st:128 * (st + 1)])
                            a_t.append(at)
                        for hcb in range(H // 512):
                            c_ps = pc.tile([128, 512], F32, tag="c")
                            for r in range(QW // 128):
                                nc.tensor.matmul(
                                    c_ps, a_t[r],
                                    wo_sb[:, r, 512 * hcb:512 * (hcb + 1)],
                                    start=(r == 0), stop=(r == QW // 128 - 1))
                            o_sb = ost.tile([128, 512], F32, tag="ostg")
                            nc.scalar.copy(o_sb, c_ps)
                            nc.sync.dma_start(
                                out_part.ap()[b, 128 * st:128 * (st + 1),
                                              512 * hcb:512 * (hcb + 1)],
                                o_sb)

    nc.compile()
    return nc


def _host_prep(hidden_states, wq, wk, wv, wo, q_norm_w, k_norm_w):
    """Build the per-core input maps (all float32 numpy)."""
    f32 = np.float32
    hsT = np.ascontiguousarray(
        np.transpose(hidden_states.astype(f32), (0, 2, 1)))

    pos = np.arange(S, dtype=np.float64)
    inv_freq = 1.0 / (THETA ** (np.arange(0, D, 2, dtype=np.float64) / D))
    freqs = pos[:, None] * inv_freq[None, :]
    emb = np.concatenate([freqs, freqs], axis=-1)           # [S, D]
    cosT = np.ascontiguousarray(np.cos(emb).T.astype(f32))  # [D, S]
    sinT = np.ascontiguousarray(np.sin(emb).T.astype(f32))

    protT = np.zeros((128, 128), f32)
    protT[64 + np.arange(64), np.arange(64)] = -1.0
    protT[np.arange(64), 64 + np.arange(64)] = 1.0

    kd = np.arange(128)[:, None]
    qd = np.arange(128)[None, :]
    mask_c = np.where(qd >= kd, 0.0, NEG).astype(f32)
    mask_w = np.where(qd < kd, 0.0, NEG).astype(f32)

    common = {
        "hsT": hsT,
        "cosT": cosT,
        "sinT": sinT,
        "ones128": np.ones((128, 1), f32),
        "ones_col": np.ones((1, 128), f32),
        "protT": protT,
        "idn": np.eye(128, dtype=f32),
        "mask_c": mask_c,
        "mask_w": mask_w,
        "wrow_q": (q_norm_w.astype(f32) / np.sqrt(D)).reshape(1, D),
        "wrow_k": k_norm_w.astype(f32).reshape(1, D),
    }
    in_maps = []
    for c in range(NCORES):
        m = dict(common)
        m["wq_s"] = np.ascontiguousarray(wq[:, QW * c:QW * (c + 1)]).astype(f32)
        m["wk_s"] = np.ascontiguousarray(wk[:, D * c:D * (c + 1)]).astype(f32)
        m["wv_s"] = np.ascontiguousarray(wv[:, D * c:D * (c + 1)]).astype(f32)
        m["wo_s"] = np.ascontiguousarray(wo[QW * c:QW * (c + 1), :]).astype(f32)
        in_maps.append(m)
    return in_maps


def kernel(hidden_states, wq, wk, wv, wo, q_norm_w, k_norm_w,
           _trace=False, _return_results=False):
    from concourse import bass_utils

    hidden_states = np.asarray(hidden_states)
    wq, wk, wv, wo = (np.asarray(a) for a in (wq, wk, wv, wo))
    q_norm_w, k_norm_w = np.asarray(q_norm_w), np.asarray(k_norm_w)

    if "nc" not in _CACHE:
        _CACHE["nc"] = _build()
    nc = _CACHE["nc"]

    in_maps = _host_prep(hidden_states, wq, wk, wv, wo, q_norm_w, k_norm_w)
    res = bass_utils.run_bass_kernel_spmd(
        nc, in_maps, core_ids=list(range(NCORES)), trace=_trace)

    out = np.zeros((B, S, H), np.float32)
    for c in range(NCORES):
        out += res.results[c]["out_part"]
    if _return_results:
        return out, res
    return out


# revision 15
# speedup vs baseline: 1.1024x; 1.1024x over previous
"""Exaone4 attention kernel for 8 Trainium2 NeuronCores.

Sharding: tensor-parallel over heads (TP=8). Core i owns query heads
4i..4i+3 and kv head i (one GQA group), processes both batch elements,
and computes a row-parallel partial of the output projection; the host
sums the 8 partials.

Shapes (hardcoded): B=2, S=2048, H=4096, NH=32, NKV=8, D=128,
WINDOW=1024, eps=1e-5, theta=10000.
"""

import os
import sys

for _p in ("/opt/trn_rl_repo",):
    if _p not in sys.path and os.path.isdir(_p):
        sys.path.insert(0, _p)

import numpy as np

B, S, H = 2, 2048, 4096
NH, NKV, D = 32, 8, 128
WINDOW = 1024
EPS = 1e-5
THETA = 10000.0

NCORES = 8
HPC = NH // NCORES          # query heads per core = 4
QW = HPC * D                # q-proj cols per core = 512
CH = 256                    # QKV sequence chunk (PSUM-bank friendly)
NCH = S // CH               # 16 chunks
HC = H // 128               # 32 contraction chunks
NEG = -1.0e30

_CACHE = {}


def _build():
    import concourse.bass as bass
    import concourse.tile as tile
    from concourse import mybir, bacc
    from concourse.tile import add_dep_helper

    F32 = mybir.dt.float32
    F32R = mybir.dt.float32r
    EXP = mybir.ActivationFunctionType.Exp
    RSQRT = mybir.ActivationFunctionType.Abs_reciprocal_sqrt
    SQUARE = mybir.ActivationFunctionType.Square

    nc = bacc.Bacc("TRN2", target_bir_lowering=False, debug=False)

    hsT = nc.dram_tensor("hsT", [B, H, S], F32R, kind="ExternalInput")
    wq_s = nc.dram_tensor("wq_s", [H, QW], F32R, kind="ExternalInput")
    wk_s = nc.dram_tensor("wk_s", [H, D], F32R, kind="ExternalInput")
    wv_s = nc.dram_tensor("wv_s", [H, D], F32R, kind="ExternalInput")
    wo_s = nc.dram_tensor("wo_s", [QW, H], F32R, kind="ExternalInput")
    cosT = nc.dram_tensor("cosT", [D, S], F32, kind="ExternalInput")
    sinT = nc.dram_tensor("sinT", [D, S], F32, kind="ExternalInput")
    wrow_q = nc.dram_tensor("wrow_q", [1, D], F32R, kind="ExternalInput")
    wrow_k = nc.dram_tensor("wrow_k", [1, D], F32R, kind="ExternalInput")
    ones128 = nc.dram_tensor("ones128", [128, 1], F32R, kind="ExternalInput")
    ones_col = nc.dram_tensor("ones_col", [1, 128], F32R, kind="ExternalInput")
    protT = nc.dram_tensor("protT", [128, 128], F32R, kind="ExternalInput")
    idn = nc.dram_tensor("idn", [128, 128], F32, kind="ExternalInput")
    mask_c = nc.dram_tensor("mask_c", [128, 128], F32, kind="ExternalInput")
    mask_w = nc.dram_tensor("mask_w", [128, 128], F32, kind="ExternalInput")
    out_part = nc.dram_tensor("out_part", [B, S, H], F32, kind="ExternalOutput")

    DEBUG = bool(os.environ.get("BASS_KERNEL_DEBUG"))
    if DEBUG:
        dbg_k = nc.dram_tensor("dbg_k", [128, S], F32, kind="ExternalOutput")
        dbg_v = nc.dram_tensor("dbg_v", [128, S // 128, 128], F32,
                               kind="ExternalOutput")
        dbg_q = nc.dram_tensor("dbg_q", [HPC, 128, S], F32,
                               kind="ExternalOutput")
        dbg_a = nc.dram_tensor("dbg_a", [HPC, 128, S], F32,
                               kind="ExternalOutput")
        dbg_raw = nc.dram_tensor("dbg_raw", [6, 128, CH], F32,
                                 kind="ExternalOutput")
        dbg_pr = nc.dram_tensor("dbg_pr", [12, 128, 512], F32,
                                kind="ExternalOutput")

    with tile.TileContext(nc) as tc, \
         nc.allow_low_precision(reason="deliberate fp32r matmul pipeline"):
        with tc.tile_pool(name="consts", bufs=1) as consts, \
             tc.tile_pool(name="dram", bufs=2, space="DRAM") as dram:
            cos_sb = consts.tile([D, S], F32)
            nc.sync.dma_start(cos_sb, cosT.ap())
            sin_sb = consts.tile([D, S], F32)
            nc.sync.dma_start(sin_sb, sinT.ap())
            wq_row = consts.tile([1, D], F32R)
            nc.sync.dma_start(wq_row, wrow_q.ap())
            wk_row = consts.tile([1, D], F32R)
            nc.sync.dma_start(wk_row, wrow_k.ap())
            on128 = consts.tile([128, 1], F32R)
            nc.sync.dma_start(on128, ones128.ap())
            oncol = consts.tile([1, 128], F32R)
            nc.sync.dma_start(oncol, ones_col.ap())
            prot = consts.tile([128, 128], F32R)
            nc.sync.dma_start(prot, protT.ap())
            iden = consts.tile([128, 128], F32)
            nc.sync.dma_start(iden, idn.ap())
            mc = consts.tile([128, 128], F32)
            nc.sync.dma_start(mc, mask_c.ap())
            mw = consts.tile([128, 128], F32)
            nc.sync.dma_start(mw, mask_w.ap())
            eps_t = consts.tile([1, 1], F32)
            nc.vector.memset(eps_t, EPS)

            scr = [dram.tile([HPC, 128, S], F32R, tag="attn_scr",
                             name=f"attn_scr{b}") for b in range(B)]

            # ---------------- phases A+B: QKV + norm/rope + attention ----
            with tc.tile_pool(name="wqkv", bufs=1) as wp, \
                 tc.tile_pool(name="kv", bufs=1) as kvp, \
                 tc.tile_pool(name="work", bufs=2) as wrk, \
                 tc.tile_pool(name="hs", bufs=3) as hsp, \
                 tc.tile_pool(name="probs", bufs=3) as prp, \
                 tc.tile_pool(name="qf", bufs=2) as qfp, \
                 tc.tile_pool(name="ps_qkv", bufs=1, space="PSUM") as pq, \
                 tc.tile_pool(name="ps_s", bufs=2, space="PSUM") as pss, \
                 tc.tile_pool(name="ps_o", bufs=1, space="PSUM") as pso, \
                 tc.tile_pool(name="ps_sum", bufs=1, space="PSUM") as psum_p, \
                 tc.tile_pool(name="ps_aux", bufs=1, space="PSUM") as paux:

                wq_sb = wp.tile([128, HC, QW], F32R)
                nc.sync.dma_start(
                    wq_sb, wq_s.ap().rearrange("(o p) c -> p o c", p=128))
                wk_sb = wp.tile([128, HC, D], F32R)
                nc.sync.dma_start(
                    wk_sb, wk_s.ap().rearrange("(o p) c -> p o c", p=128))
                wv_sb = wp.tile([128, HC, D], F32R)
                nc.sync.dma_start(
                    wv_sb, wv_s.ap().rearrange("(o p) c -> p o c", p=128))

                for b in range(B):
                    # K in [D, S]; V in [S, D] (by 128-row tiles)
                    k_full = kvp.tile([128, S], F32R, tag="k_full")
                    v_full = kvp.tile([128, S // 128, 128], F32R, tag="v_full")

                    for qc in range(S // 512):
                        qfin = []
                        for half in range(2):
                            sc2 = 2 * qc + half
                            s0 = CH * sc2
                            # ---- QKV projection for this 256-chunk
                            # NOTE: a matmul with start=True clears the
                            # has_written bits of the WHOLE psum bank, so for
                            # the two 256-wide blocks sharing a bank only the
                            # even block seeds (start=True) and the odd block
                            # must issue after it at hc==0 (dep edge below).
                            qkv_ps = pq.tile([128, 6, CH], F32, tag="qkv")
                            seed_mm = {}
                            for hc in range(HC):
                                ht = hsp.tile([128, CH], F32R, tag="hst")
                                nc.sync.dma_start(
                                    ht, hsT.ap()[b, 128 * hc:128 * (hc + 1),
                                                 s0:s0 + CH])
                                for blk in range(6):
                                    if blk < HPC:
                                        lhs = wq_sb[:, hc,
                                                    128 * blk:128 * (blk + 1)]
                                    elif blk == HPC:
                                        lhs = wk_sb[:, hc, :]
                                    else:
                                        lhs = wv_sb[:, hc, :]
                                    mm = nc.tensor.matmul(
                                        qkv_ps[:, blk, :], lhs, ht,
                                        start=(hc == 0 and blk % 2 == 0),
                                        stop=(hc == HC - 1),
                                        skip_group_check=True)
                                    if hc == 0:
                                        if blk % 2 == 0:
                                            seed_mm[blk // 2] = mm.ins
                                        else:
                                            add_dep_helper(
                                                mm.ins, seed_mm[blk // 2],
                                                sync=False,
                                                reason="psum bank seed order")

                            # ---- norm + rope for the 4 q blocks and k
                            for blk in range(HPC + 1):
                                is_q = blk < HPC
                                raw = wrk.tile([128, CH], F32, tag="raw")
                                nc.scalar.copy(raw, qkv_ps[:, blk, :])
                                sq = wrk.tile([128, CH], F32R, tag="sq")
                                nc.vector.tensor_mul(sq, raw, raw)
                                ssq = paux.tile([1, CH], F32, tag="aux")
                                nc.tensor.matmul(ssq, on128, sq,
                                                 start=True, stop=True)
                                rinv = wrk.tile([1, CH], F32R, tag="rinv")
                                nc.scalar.activation(rinv, ssq, RSQRT,
                                                     bias=eps_t, scale=1.0 / D)
                                wrow = wq_row if is_q else wk_row
                                wtil = paux.tile([128, CH], F32, tag="aux")
                                nc.tensor.matmul(wtil, wrow, rinv,
                                                 start=True, stop=True)
                                qhat = wrk.tile([128, CH], F32R, tag="qhat")
                                nc.vector.tensor_mul(qhat, wtil, raw)
                                rot = paux.tile([128, CH], F32, tag="aux")
                                nc.tensor.matmul(rot, prot, qhat,
                                                 start=True, stop=True)
                                t1 = wrk.tile([128, CH], F32, tag="t1")
                                nc.vector.tensor_mul(t1, qhat,
                                                     cos_sb[:, s0:s0 + CH])
                                t2 = wrk.tile([128, CH], F32, tag="t2")
                                nc.vector.tensor_mul(t2, rot,
                                                     sin_sb[:, s0:s0 + CH])
                                if is_q:
                                    if half == 0:
                                        qf = qfp.tile([128, 512], F32R,
                                                      tag=f"qfin{blk}")
                                        qfin.append(qf)
                                    nc.vector.tensor_add(
                                        qfin[blk][:, CH * half:CH * (half + 1)],
                                        t1, t2)
                                else:
                                    nc.vector.tensor_add(
                                        k_full[:, s0:s0 + CH], t1, t2)
                                if DEBUG and b == 0 and sc2 == 4:
                                    nc.sync.dma_start(
                                        dbg_raw.ap()[blk], raw)

                            # ---- V: copy out and transpose to [S, D]
                            vt_sb = wrk.tile([128, CH], F32, tag="vt")
                            nc.scalar.copy(vt_sb, qkv_ps[:, HPC + 1, :])
                            for j in range(CH // 128):
                                tp = paux.tile([128, 128], F32, tag="aux")
                                nc.tensor.transpose(
                                    tp, vt_sb[:, 128 * j:128 * (j + 1)], iden)
                                nc.scalar.copy(
                                    v_full[:, 2 * sc2 + j, :], tp)

                        # ---- attention for query chunk qc (512 queries)
                        s0q = 512 * qc
                        kis = list(range(max(0, 4 * qc - 8), 4 * qc + 4))
                        for h in range(HPC):
                            o_ps = pso.tile([128, 512], F32, tag="o")
                            sum_ps = psum_p.tile([1, 512], F32, tag="sum")
                            for i, ki in enumerate(kis):
                                first, last = (i == 0), (i == len(kis) - 1)
                                s_ps = pss.tile([128, 512], F32, tag="s")
                                nc.tensor.matmul(
                                    s_ps,
                                    k_full[:, 128 * ki:128 * (ki + 1)],
                                    qfin[h], start=True, stop=True)
                                delta = 512 * qc - 128 * ki
                                j = None
                                if delta <= 0:
                                    j = -delta // 128
                                    nc.vector.tensor_add(
                                        s_ps[:, 128 * j:128 * (j + 1)],
                                        s_ps[:, 128 * j:128 * (j + 1)], mc)
                                elif delta >= 640:
                                    j = (1024 - delta) // 128
                                    nc.vector.tensor_add(
                                        s_ps[:, 128 * j:128 * (j + 1)],
                                        s_ps[:, 128 * j:128 * (j + 1)], mw)
                                pr = prp.tile([128, 512], F32R, tag="pr")
                                nc.scalar.activation(pr, s_ps, EXP)
                                if delta <= 0 and j is not None and j > 0:
                                    nc.gpsimd.memset(
                                        pr[:, :128 * j].bitcast(F32), 0.0)
                                if delta >= 640 and j is not None and j < 3:
                                    nc.gpsimd.memset(
                                        pr[:, 128 * (j + 1):].bitcast(F32),
                                        0.0)
                                if DEBUG and b == 0 and h == 0 and qc == 2:
                                    nc.sync.dma_start(
                                        dbg_pr.ap()[i], pr.bitcast(F32))
                                nc.tensor.matmul(sum_ps, on128, pr,
                                                 start=first, stop=last)
                                nc.tensor.matmul(o_ps, v_full[:, ki, :], pr,
                                                 start=first, stop=last)
                            rsq = wrk.tile([1, 512], F32, tag="rsq")
                            nc.scalar.activation(rsq, sum_ps, RSQRT)
                            rsum = wrk.tile([1, 512], F32R, tag="rsum")
                            nc.scalar.activation(rsum, rsq, SQUARE)
                            bc = paux.tile([128, 512], F32, tag="aux")
                            nc.tensor.matmul(bc, oncol, rsum,
                                             start=True, stop=True)
                            a_un = wrk.tile([128, 512], F32, tag="a_un")
                            nc.scalar.copy(a_un, o_ps)
                            a_fin = wrk.tile([128, 512], F32R, tag="a_fin")
                            nc.vector.tensor_mul(a_fin, bc, a_un)
                            nc.sync.dma_start(
                                scr[b][h, :, s0q:s0q + 512], a_fin)
                            if DEBUG and b == 0:
                                nc.sync.dma_start(
                                    dbg_a.ap()[h, :, s0q:s0q + 512],
                                    a_fin.bitcast(F32))
                                nc.sync.dma_start(
                                    dbg_q.ap()[h, :, s0q:s0q + 512],
                                    qfin[h].bitcast(F32))

                    if DEBUG and b == 0:
                        nc.sync.dma_start(dbg_k.ap(), k_full.bitcast(F32))
                        nc.sync.dma_start(dbg_v.ap(), v_full.bitcast(F32))

            # ---------------- phase C: output projection -----------------
            with tc.tile_pool(name="wo", bufs=1) as wop, \
                 tc.tile_pool(name="at", bufs=4) as atp, \
                 tc.tile_pool(name="ostg", bufs=4) as ost, \
                 tc.tile_pool(name="ps_c", bufs=4, space="PSUM") as pc:
                wo_sb = wop.tile([128, QW // 128, H], F32R)
                nc.sync.dma_start(
                    wo_sb, wo_s.ap().rearrange("(o p) c -> p o c", p=128))
                for b in range(B):
                    for st in range(S // 128):
                        a_t = []
                        for r in range(QW // 128):
                            at = atp.tile([128, 128], F32R, tag=f"at{r}")
                            nc.sync.dma_start(
                                at, scr[b][r, :, 128 * st:128 * (st + 1)])
                            a_t.append(at)
                        for hcb in range(H // 512):
                            c_ps = pc.tile([128, 512], F32, tag="c")
                            for r in range(QW // 128):
                                nc.tensor.matmul(
                                    c_ps, a_t[r],
                                    wo_sb[:, r, 512 * hcb:512 * (hcb + 1)],
                                    start=(r == 0), stop=(r == QW // 128 - 1))
                            o_sb = ost.tile([128, 512], F32, tag="ostg")
                            nc.scalar.copy(o_sb, c_ps)
                            nc.sync.dma_start(
                                out_part.ap()[b, 128 * st:128 * (st + 1),
                                              512 * hcb:512 * (hcb + 1)],
                                o_sb)

    nc.compile()
    return nc


def _host_prep(hidden_states, wq, wk, wv, wo, q_norm_w, k_norm_w):
    """Build the per-core input maps (all float32 numpy)."""
    f32 = np.float32
    hsT = np.ascontiguousarray(
        np.transpose(hidden_states.astype(f32), (0, 2, 1)))

    pos = np.arange(S, dtype=np.float64)
    inv_freq = 1.0 / (THETA ** (np.arange(0, D, 2, dtype=np.float64) / D))
    freqs = pos[:, None] * inv_freq[None, :]
    emb = np.concatenate([freqs, freqs], axis=-1)           # [S, D]
    cosT = np.ascontiguousarray(np.cos(emb).T.astype(f32))  # [D, S]
    sinT = np.ascontiguousarray(np.sin(emb).T.astype(f32))

    protT = np.zeros((128, 128), f32)
    protT[64 + np.arange(64), np.arange(64)] = -1.0
    protT[np.arange(64), 64 + np.arange(64)] = 1.0

    kd = np.arange(128)[:, None]
    qd = np.arange(128)[None, :]
    mask_c = np.where(qd >= kd, 0.0, NEG).astype(f32)
    mask_w = np.where(qd < kd, 0.0, NEG).astype(f32)

    common = {
        "hsT": hsT,
        "cosT": cosT,
        "sinT": sinT,
        "ones128": np.ones((128, 1), f32),
        "ones_col": np.ones((1, 128), f32),
        "protT": protT,
        "idn": np.eye(128, dtype=f32),
        "mask_c": mask_c,
        "mask_w": mask_w,
        "wrow_q": (q_norm_w.astype(f32) / np.sqrt(D)).reshape(1, D),
        "wrow_k": k_norm_w.astype(f32).reshape(1, D),
    }
    in_maps = []
    for c in range(NCORES):
        m = dict(common)
        m["wq_s"] = np.ascontiguousarray(wq[:, QW * c:QW * (c + 1)]).astype(f32)
        m["wk_s"] = np.ascontiguousarray(wk[:, D * c:D * (c + 1)]).astype(f32)
        m["wv_s"] = np.ascontiguousarray(wv[:, D * c:D * (c + 1)]).astype(f32)
        m["wo_s"] = np.ascontiguousarray(wo[QW * c:QW * (c + 1), :]).astype(f32)
        in_maps.append(m)
    return in_maps


def kernel(hidden_states, wq, wk, wv, wo, q_norm_w, k_norm_w,
           _trace=False, _return_results=False):
    from concourse import bass_utils

    hidden_states = np.asarray(hidden_states)
    wq, wk, wv, wo = (np.asarray(a) for a in (wq, wk, wv, wo))
    q_norm_w, k_norm_w = np.asarray(q_norm_w), np.asarray(k_norm_w)

    if "nc" not in _CACHE:
        _CACHE["nc"] = _build()
    nc = _CACHE["nc"]

    in_maps = _host_prep(hidden_states, wq, wk, wv, wo, q_norm_w, k_norm_w)
    res = bass_utils.run_bass_kernel_spmd(
        nc, in_maps, core_ids=list(range(NCORES)), trace=_trace)

    out = np.zeros((B, S, H), np.float32)
    for c in range(NCORES):
        out += res.results[c]["out_part"]
    if _return_results:
        return out, res
    return out
